# revision 7
# baseline (speedup 1.0000x reference)
"""GCN layer (gather + segment-mean + linear) as a Bass/Tile kernel on 8 TRN2 cores.

Strategy (edge/data parallel, sharded by destination node):
  - Each core owns a contiguous range of N/8 destination nodes and processes
    exactly the edges landing in that range, so no cross-core reduction is
    needed; each core emits its own output slice.
  - h[src] rows are gathered from DRAM with GPSIMD dma_gather (512B rows).
    dma_gather indices are int16, so each 128-node group's edges are split
    into a "lo" run (src < 32768, gathered from h[0:32768]) and a "hi" run
    (src >= 32768, gathered from a view of h offset by 32768 rows).
  - Segment-sum over a 128-edge chunk is one PE matmul: S.T @ [hg|r] where
    S[e, n] = (dst_local[e] == n) is built on DVE from an iota row and the
    per-edge dst value.  PSUM accumulates [node, 256] per 128-node group
    (left half: sum of gathered h, right half: sum of r).
  - Epilogue per group: add halves, scale rows by 1/max(indeg,1)
    (host-precomputed), transpose on PE, GEMM with W^T, add bias, DMA out.
  - One SPMD program shared by all cores: per-(group, half) chunk counts are
    padded up to the max over the 8 cores; pad edges carry dst=-1 so their
    one-hot column is zero and they contribute nothing.
"""

import numpy as np

N_NODES = 50000
D = 128
N_CORES = 8
HALF = 32768  # int16 index reach for dma_gather


def _preprocess(src, dst, h, r, W, b, n_cores=N_CORES, n_nodes=N_NODES, half=HALF):
    src = np.asarray(src).astype(np.int64)
    dst = np.asarray(dst).astype(np.int64)
    h = np.ascontiguousarray(np.asarray(h, dtype=np.float32))
    r = np.asarray(r, dtype=np.float32)
    W = np.asarray(W, dtype=np.float32)
    b = np.asarray(b, dtype=np.float32)
    E = src.shape[0]
    npc = n_nodes // n_cores
    G = -(-npc // 128)

    counts = np.bincount(dst, minlength=n_nodes).astype(np.float32)
    inv = (1.0 / np.maximum(counts, 1.0)).astype(np.float32)

    core = dst // npc
    nl = dst % npc
    g = nl // 128
    is_hi = (src >= half).astype(np.int64)
    key = (core * G + g) * 2 + is_hi
    nkeys = n_cores * G * 2

    cnt = np.bincount(key, minlength=nkeys)
    chunks = -(-cnt // 128)
    caps = chunks.reshape(n_cores, G, 2).max(axis=0)  # [G, 2] chunks per run
    caps = np.maximum(caps, 1)

    run_chunk_base = np.zeros(G * 2, np.int64)
    flat = caps.reshape(-1)
    run_chunk_base[1:] = np.cumsum(flat)[:-1]
    run_chunk_base = run_chunk_base.reshape(G, 2)
    total_chunks = int(flat.sum())
    P_edges = total_chunks * 128

    order = np.argsort(key, kind="stable")
    key_s = key[order]
    run_starts = np.zeros(nkeys, np.int64)
    cnt_cum = np.cumsum(cnt)
    run_starts[1:] = cnt_cum[:-1]
    offs = np.arange(E, dtype=np.int64) - run_starts[key_s]
    ecore = key_s // (G * 2)
    eg = (key_s // 2) % G
    ehalf = key_s % 2
    pos = run_chunk_base[eg, ehalf] * 128 + offs  # position in the padded stream

    per_core = []
    for c in range(n_cores):
        m = ecore == c
        p = pos[m]
        e = order[m]
        rT = np.zeros((128, total_chunks, 128), np.float32)
        rT[p % 128, p // 128, :] = r[e]
        idx16 = np.zeros(P_edges, np.int16)
        idx16[p] = (src[e] - half * ehalf[m]).astype(np.int16)
        dstf = np.full((128, total_chunks), -1.0, np.float32)
        dstf[p % 128, p // 128] = (nl[e] - eg[m] * 128).astype(np.float32)
        idxw = np.tile(np.ascontiguousarray(idx16.reshape(-1, 16).T), (8, 1))
        invp = np.zeros(G * 128, np.float32)
        invp[:npc] = inv[c * npc : (c + 1) * npc]
        inv_t = np.ascontiguousarray(invp.reshape(G, 128).T)
        per_core.append(
            {
                "h": h,
                "rT": rT.reshape(128, total_chunks * 128),
                "idxw": np.ascontiguousarray(idxw),
                "dstf": dstf,
                "invt": inv_t,
                "iota": np.tile(np.arange(128, dtype=np.float32), (128, 1)),
                "wt": np.ascontiguousarray(W.T),
                "bb": np.tile(b, (128, 1)),
                "ident": np.eye(128, dtype=np.float32),
            }
        )
    return per_core, caps, total_chunks, npc, G


def _build(caps, total_chunks, npc, G, n_nodes=N_NODES, half=HALF, gmax=8):
    from contextlib import ExitStack

    import concourse.bacc as bacc
    import concourse.mybir as mybir
    import concourse.tile as tile

    f32 = mybir.dt.float32
    i16 = mybir.dt.int16
    ADD = mybir.AluOpType.add
    MUL = mybir.AluOpType.mult
    ISEQ = mybir.AluOpType.is_equal

    Bmax = int(caps.max())
    Gmax = int(caps.sum(axis=1).max())

    nc = bacc.Bacc("TRN2", target_bir_lowering=False, debug=False)
    h_d = nc.dram_tensor("h", [n_nodes, 128], f32, kind="ExternalInput")
    r_d = nc.dram_tensor("rT", [128, total_chunks * 128], f32, kind="ExternalInput")
    idx_d = nc.dram_tensor("idxw", [128, total_chunks * 8], i16, kind="ExternalInput")
    dstf_d = nc.dram_tensor("dstf", [128, total_chunks], f32, kind="ExternalInput")
    inv_d = nc.dram_tensor("invt", [128, G], f32, kind="ExternalInput")
    iota_d = nc.dram_tensor("iota", [128, 128], f32, kind="ExternalInput")
    wt_d = nc.dram_tensor("wt", [128, 128], f32, kind="ExternalInput")
    bb_d = nc.dram_tensor("bb", [128, 128], f32, kind="ExternalInput")
    id_d = nc.dram_tensor("ident", [128, 128], f32, kind="ExternalInput")
    out_d = nc.dram_tensor("out", [npc, 128], f32, kind="ExternalOutput")

    h_lo = h_d[0:half, :]
    h_hi = h_d[half:n_nodes, :]

    with tile.TileContext(nc) as tc, ExitStack() as ctx:
        const = ctx.enter_context(tc.tile_pool(name="const", bufs=1))
        datap = ctx.enter_context(tc.tile_pool(name="data", bufs=3))
        idxp = ctx.enter_context(tc.tile_pool(name="idx", bufs=2))
        dstp = ctx.enter_context(tc.tile_pool(name="dstf", bufs=2))
        sp = ctx.enter_context(tc.tile_pool(name="s", bufs=4))
        ftp = ctx.enter_context(tc.tile_pool(name="ft", bufs=2))
        outp = ctx.enter_context(tc.tile_pool(name="o", bufs=2))
        psA = ctx.enter_context(tc.tile_pool(name="psA", bufs=2, space="PSUM"))
        psT = ctx.enter_context(tc.tile_pool(name="psT", bufs=2, space="PSUM"))
        psO = ctx.enter_context(tc.tile_pool(name="psO", bufs=2, space="PSUM"))

        iota_t = const.tile([128, 128], f32)
        nc.sync.dma_start(iota_t[:], iota_d[:])
        wt_t = const.tile([128, 128], f32)
        nc.sync.dma_start(wt_t[:], wt_d[:])
        bb_t = const.tile([128, 128], f32)
        nc.sync.dma_start(bb_t[:], bb_d[:])
        id_t = const.tile([128, 128], f32)
        nc.sync.dma_start(id_t[:], id_d[:])
        inv_t = const.tile([128, G], f32)
        nc.sync.dma_start(inv_t[:], inv_d[:])

        cb = 0
        for g in range(G):
            ncl, nch = int(caps[g, 0]), int(caps[g, 1])
            ng = ncl + nch
            ix = idxp.tile([128, Gmax * 8], i16)
            nc.sync.dma_start(ix[:, : ng * 8], idx_d[:, cb * 8 : (cb + ng) * 8])
            df = dstp.tile([128, Gmax], f32)
            nc.sync.dma_start(df[:, :ng], dstf_d[:, cb : cb + ng])
            acc = psA.tile([128, 128], f32)
            done = 0
            for hf, ncap in ((0, ncl), (1, nch)):
                if ncap == 0:
                    continue
                bt = datap.tile([128, 2, Bmax, 128], f32)
                for q0 in range(0, ncap, gmax):
                    qn = min(gmax, ncap - q0)
                    nc.gpsimd.dma_gather(
                        bt[:, 0, q0 : q0 + qn, :],
                        h_lo if hf == 0 else h_hi,
                        ix[:, (done + q0) * 8 : (done + q0 + qn) * 8],
                        qn * 128,
                        qn * 128,
                        128,
                    )
                nc.sync.dma_start(
                    bt[:, 1, 0:ncap, :],
                    r_d[:, (cb + done) * 128 : (cb + done + ncap) * 128].rearrange(
                        "p (j f) -> p j f", f=128
                    ),
                )
                for j in range(ncap):
                    S = sp.tile([128, 128], f32)
                    nc.vector.tensor_scalar(
                        S[:], iota_t[:], df[:, done + j : done + j + 1], None, ISEQ
                    )
                    nc.vector.tensor_tensor(
                        bt[:, 1, j, :], bt[:, 0, j, :], bt[:, 1, j, :], ADD
                    )
                    nc.tensor.matmul(
                        acc[:],
                        S[:],
                        bt[:, 1, j, :],
                        start=(done + j == 0),
                        stop=(done + j == ng - 1),
                    )
                done += ncap
            ft = ftp.tile([128, 128], f32)
            nc.vector.tensor_scalar(ft[:], acc[:], inv_t[:, g : g + 1], None, MUL)
            pt = psT.tile([128, 128], f32)
            nc.tensor.transpose(pt[:], ft[:], id_t[:])
            ftT = ftp.tile([128, 128], f32)
            nc.vector.tensor_copy(ftT[:], pt[:])
            po = psO.tile([128, 128], f32)
            nc.tensor.matmul(po[:], ftT[:], wt_t[:], start=True, stop=True)
            ot = outp.tile([128, 128], f32)
            nc.vector.tensor_tensor(ot[:], po[:], bb_t[:], ADD)
            rows = min(128, npc - g * 128)
            nc.sync.dma_start(out_d[g * 128 : g * 128 + rows, :], ot[0:rows, :])
            cb += ng

    nc.compile()
    return nc


LAST_RESULT = None


def kernel(src, dst, h, r, W, b, _trace=False, _tmpdir=None):
    global LAST_RESULT
    from concourse.bass_utils import run_bass_kernel_spmd

    per_core, caps, total_chunks, npc, G = _preprocess(src, dst, h, r, W, b)
    nc = _build(caps, total_chunks, npc, G)
    kwargs = {}
    if _trace:
        kwargs = dict(trace=True, tmpdir=_tmpdir)
    res = run_bass_kernel_spmd(nc, per_core, list(range(N_CORES)), **kwargs)
    LAST_RESULT = res
    out = np.concatenate([res.results[c]["out"] for c in range(N_CORES)], axis=0)
    return out.astype(np.float32)


# revision 12
# speedup vs baseline: 1.0551x; 1.0551x over previous
"""GCN layer (gather + segment-mean + linear) as a Bass/Tile kernel on 8 TRN2 cores.

Strategy (edge/data parallel, sharded by destination node):
  - Each core owns a contiguous range of N/8 destination nodes and processes
    exactly the edges landing in that range, so no cross-core reduction is
    needed; each core emits its own output slice.
  - h[src] rows are gathered from DRAM with GPSIMD dma_gather (512B rows).
    dma_gather indices are int16, so each 128-node group's edges are split
    into a "lo" run (src < 32768, gathered from h[0:32768]) and a "hi" run
    (src >= 32768, gathered from a view of h offset by 32768 rows).
  - Segment-sum over a 128-edge chunk is one PE matmul: S.T @ [hg|r] where
    S[e, n] = (dst_local[e] == n) is built on DVE from an iota row and the
    per-edge dst value.  PSUM accumulates [node, 256] per 128-node group
    (left half: sum of gathered h, right half: sum of r).
  - Epilogue per group: add halves, scale rows by 1/max(indeg,1)
    (host-precomputed), transpose on PE, GEMM with W^T, add bias, DMA out.
  - One SPMD program shared by all cores: per-(group, half) chunk counts are
    padded up to the max over the 8 cores; pad edges carry dst=-1 so their
    one-hot column is zero and they contribute nothing.
"""

import numpy as np

N_NODES = 50000
D = 128
N_CORES = 8
HALF = 32768  # int16 index reach for dma_gather


def _preprocess(src, dst, h, r, W, b, n_cores=N_CORES, n_nodes=N_NODES, half=HALF):
    src = np.asarray(src).astype(np.int64)
    dst = np.asarray(dst).astype(np.int64)
    h = np.ascontiguousarray(np.asarray(h, dtype=np.float32))
    r = np.asarray(r, dtype=np.float32)
    W = np.asarray(W, dtype=np.float32)
    b = np.asarray(b, dtype=np.float32)
    E = src.shape[0]
    npc = n_nodes // n_cores
    G = -(-npc // 128)

    counts = np.bincount(dst, minlength=n_nodes).astype(np.float32)
    inv = (1.0 / np.maximum(counts, 1.0)).astype(np.float32)

    core = dst // npc
    nl = dst % npc
    g = nl // 128
    is_hi = (src >= half).astype(np.int64)
    key = (core * G + g) * 2 + is_hi
    nkeys = n_cores * G * 2

    cnt = np.bincount(key, minlength=nkeys)
    chunks = -(-cnt // 128)
    caps = chunks.reshape(n_cores, G, 2).max(axis=0)  # [G, 2] chunks per run
    caps = np.maximum(caps, 1)

    run_chunk_base = np.zeros(G * 2, np.int64)
    flat = caps.reshape(-1)
    run_chunk_base[1:] = np.cumsum(flat)[:-1]
    run_chunk_base = run_chunk_base.reshape(G, 2)
    total_chunks = int(flat.sum())
    P_edges = total_chunks * 128

    order = np.argsort(key, kind="stable")
    key_s = key[order]
    run_starts = np.zeros(nkeys, np.int64)
    cnt_cum = np.cumsum(cnt)
    run_starts[1:] = cnt_cum[:-1]
    offs = np.arange(E, dtype=np.int64) - run_starts[key_s]
    ecore = key_s // (G * 2)
    eg = (key_s // 2) % G
    ehalf = key_s % 2
    pos = run_chunk_base[eg, ehalf] * 128 + offs  # position in the padded stream

    per_core = []
    for c in range(n_cores):
        m = ecore == c
        p = pos[m]
        e = order[m]
        rT = np.zeros((128, total_chunks, 128), np.float32)
        rT[p % 128, p // 128, :] = r[e]
        idx16 = np.zeros(P_edges, np.int16)
        idx16[p] = (src[e] - half * ehalf[m]).astype(np.int16)
        dstf = np.full((128, total_chunks), -1.0, np.float32)
        dstf[p % 128, p // 128] = (nl[e] - eg[m] * 128).astype(np.float32)
        idxw = np.tile(np.ascontiguousarray(idx16.reshape(-1, 16).T), (8, 1))
        invp = np.zeros(G * 128, np.float32)
        invp[:npc] = inv[c * npc : (c + 1) * npc]
        inv_t = np.ascontiguousarray(invp.reshape(G, 128).T)
        per_core.append(
            {
                "h": h,
                "rT": rT.reshape(128, total_chunks * 128),
                "idxw": np.ascontiguousarray(idxw),
                "dstf": dstf,
                "invt": inv_t,
                "iota": np.tile(np.arange(128, dtype=np.float32), (128, 1)),
                "wt": np.ascontiguousarray(W.T),
                "bb": np.tile(b, (128, 1)),
                "ident": np.eye(128, dtype=np.float32),
            }
        )
    return per_core, caps, total_chunks, npc, G


def _build(caps, total_chunks, npc, G, n_nodes=N_NODES, half=HALF, gmax=8, nq=4):
    from contextlib import ExitStack

    import concourse.bacc as bacc
    import concourse.mybir as mybir
    import concourse.tile as tile

    f32 = mybir.dt.float32
    i16 = mybir.dt.int16
    ADD = mybir.AluOpType.add
    MUL = mybir.AluOpType.mult
    ISEQ = mybir.AluOpType.is_equal

    Bmax = int(caps.max())
    Gmax = int(caps.sum(axis=1).max())

    nc = bacc.Bacc(
        "TRN2", target_bir_lowering=False, debug=False, num_swdge_queues=nq
    )
    h_d = nc.dram_tensor("h", [n_nodes, 128], f32, kind="ExternalInput")
    r_d = nc.dram_tensor("rT", [128, total_chunks * 128], f32, kind="ExternalInput")
    idx_d = nc.dram_tensor("idxw", [128, total_chunks * 8], i16, kind="ExternalInput")
    dstf_d = nc.dram_tensor("dstf", [128, total_chunks], f32, kind="ExternalInput")
    inv_d = nc.dram_tensor("invt", [128, G], f32, kind="ExternalInput")
    iota_d = nc.dram_tensor("iota", [128, 128], f32, kind="ExternalInput")
    wt_d = nc.dram_tensor("wt", [128, 128], f32, kind="ExternalInput")
    bb_d = nc.dram_tensor("bb", [128, 128], f32, kind="ExternalInput")
    id_d = nc.dram_tensor("ident", [128, 128], f32, kind="ExternalInput")
    out_d = nc.dram_tensor("out", [npc, 128], f32, kind="ExternalOutput")

    h_lo = h_d[0:half, :]
    h_hi = h_d[half:n_nodes, :]

    with tile.TileContext(nc) as tc, ExitStack() as ctx:
        const = ctx.enter_context(tc.tile_pool(name="const", bufs=1))
        datap = ctx.enter_context(tc.tile_pool(name="data", bufs=8))
        idxp = ctx.enter_context(tc.tile_pool(name="idx", bufs=2))
        dstp = ctx.enter_context(tc.tile_pool(name="dstf", bufs=2))
        sp = ctx.enter_context(tc.tile_pool(name="s", bufs=4))
        ftp = ctx.enter_context(tc.tile_pool(name="ft", bufs=2))
        outp = ctx.enter_context(tc.tile_pool(name="o", bufs=2))
        psA = ctx.enter_context(tc.tile_pool(name="psA", bufs=2, space="PSUM"))
        psT = ctx.enter_context(tc.tile_pool(name="psT", bufs=2, space="PSUM"))
        psO = ctx.enter_context(tc.tile_pool(name="psO", bufs=2, space="PSUM"))

        iota_t = const.tile([128, 128], f32)
        nc.sync.dma_start(iota_t[:], iota_d[:])
        wt_t = const.tile([128, 128], f32)
        nc.sync.dma_start(wt_t[:], wt_d[:])
        bb_t = const.tile([128, 128], f32)
        nc.sync.dma_start(bb_t[:], bb_d[:])
        id_t = const.tile([128, 128], f32)
        nc.sync.dma_start(id_t[:], id_d[:])
        inv_t = const.tile([128, G], f32)
        nc.sync.dma_start(inv_t[:], inv_d[:])

        cb = 0
        qctr = 0
        for g in range(G):
            ncl, nch = int(caps[g, 0]), int(caps[g, 1])
            ng = ncl + nch
            ix = idxp.tile([128, Gmax * 8], i16)
            nc.sync.dma_start(ix[:, : ng * 8], idx_d[:, cb * 8 : (cb + ng) * 8])
            df = dstp.tile([128, Gmax], f32)
            nc.sync.dma_start(df[:, :ng], dstf_d[:, cb : cb + ng])
            acc = psA.tile([128, 128], f32)
            done = 0
            for hf, ncap in ((0, ncl), (1, nch)):
                if ncap == 0:
                    continue
                hsrc = h_lo if hf == 0 else h_hi
                for q0 in range(0, ncap, gmax):
                    qn = min(gmax, ncap - q0)
                    c0 = done + q0
                    bt = datap.tile([128, gmax, 128], f32)
                    nc.gpsimd.dma_gather(
                        bt[:, 0:qn, :],
                        hsrc,
                        ix[:, c0 * 8 : (c0 + qn) * 8],
                        qn * 128,
                        qn * 128,
                        128,
                        queue_num=qctr % nq,
                    )
                    qctr += 1
                    nc.gpsimd.dma_start(
                        bt[:, 0:qn, :],
                        r_d[:, (cb + c0) * 128 : (cb + c0 + qn) * 128].rearrange(
                            "p (j f) -> p j f", f=128
                        ),
                        accum_op=ADD,
                    )
                    for j in range(qn):
                        S = sp.tile([128, 128], f32)
                        nc.vector.tensor_scalar(
                            S[:], iota_t[:], df[:, c0 + j : c0 + j + 1], None, ISEQ
                        )
                        nc.tensor.matmul(
                            acc[:],
                            S[:],
                            bt[:, j, :],
                            start=(c0 + j == 0),
                            stop=(c0 + j == ng - 1),
                        )
                done += ncap
            ft = ftp.tile([128, 128], f32)
            nc.vector.tensor_scalar(ft[:], acc[:], inv_t[:, g : g + 1], None, MUL)
            pt = psT.tile([128, 128], f32)
            nc.tensor.transpose(pt[:], ft[:], id_t[:])
            ftT = ftp.tile([128, 128], f32)
            nc.vector.tensor_copy(ftT[:], pt[:])
            po = psO.tile([128, 128], f32)
            nc.tensor.matmul(po[:], ftT[:], wt_t[:], start=True, stop=True)
            ot = outp.tile([128, 128], f32)
            nc.vector.tensor_tensor(ot[:], po[:], bb_t[:], ADD)
            rows = min(128, npc - g * 128)
            nc.sync.dma_start(out_d[g * 128 : g * 128 + rows, :], ot[0:rows, :])
            cb += ng

    nc.compile()
    return nc


LAST_RESULT = None


def kernel(src, dst, h, r, W, b, _trace=False, _tmpdir=None):
    global LAST_RESULT
    from concourse.bass_utils import run_bass_kernel_spmd

    per_core, caps, total_chunks, npc, G = _preprocess(src, dst, h, r, W, b)
    nc = _build(caps, total_chunks, npc, G)
    kwargs = {}
    if _trace:
        kwargs = dict(trace=True, tmpdir=_tmpdir)
    res = run_bass_kernel_spmd(nc, per_core, list(range(N_CORES)), **kwargs)
    LAST_RESULT = res
    out = np.concatenate([res.results[c]["out"] for c in range(N_CORES)], axis=0)
    return out.astype(np.float32)


# revision 15
# speedup vs baseline: 1.8850x; 1.7866x over previous
"""GCN layer (gather + segment-mean + linear) as a Bass/Tile kernel on 8 TRN2 cores.

Strategy (edge parallel, sharded by destination node):
  - Each core owns a contiguous range of N/8 destination nodes and processes
    exactly the edges landing in that range; no cross-core reduction needed.
  - h and r are carried as double-bf16 (hi + lo) pairs, exact to ~2^-16
    relative: h_split [N, 256] bf16 rows are 512B, gathered per edge with
    GPSIMD dma_gather.  dma_gather indices are int16, so each 128-node
    group's edges are split into a "lo" run (src < 32768) and a "hi" run
    (gathered from a view of h_split offset by 32768 rows).
  - r_split streams via HWDGE (sync engine), keeping the GPSIMD engine
    dedicated to gather descriptor generation (the kernel's critical path).
  - Segment-sum over a 128-edge chunk is one bf16 PE matmul: S.T @
    [hg_hi|hg_lo|r_hi|r_lo] where S[e, n] = (dst_local[e] == n) is built on
    DVE from an iota row; bf16 one-hot entries are exact.  PSUM accumulates
    [node, 512] f32 per 128-node group.
  - Epilogue per group: sum the four psum quarters, scale rows by
    1/max(indeg,1) (host-precomputed), transpose on PE, GEMM with W^T (f32),
    add bias, DMA out.
  - One SPMD program shared by all cores: per-(group, half) chunk counts are
    padded up to the max over the 8 cores; pad edges carry dst=-1 so their
    one-hot column is zero and they contribute nothing.
"""

import numpy as np

N_NODES = 50000
D = 128
N_CORES = 8
HALF = 32768  # int16 index reach for dma_gather


def _bf16_split(x):
    import ml_dtypes

    hi = x.astype(ml_dtypes.bfloat16)
    lo = (x - hi.astype(np.float32)).astype(ml_dtypes.bfloat16)
    return hi, lo


def _preprocess(src, dst, h, r, W, b, n_cores=N_CORES, n_nodes=N_NODES, half=HALF):
    import ml_dtypes

    bf16 = ml_dtypes.bfloat16
    src = np.asarray(src).astype(np.int64)
    dst = np.asarray(dst).astype(np.int64)
    h = np.ascontiguousarray(np.asarray(h, dtype=np.float32))
    r = np.asarray(r, dtype=np.float32)
    W = np.asarray(W, dtype=np.float32)
    b = np.asarray(b, dtype=np.float32)
    E = src.shape[0]
    npc = n_nodes // n_cores
    G = -(-npc // 128)

    counts = np.bincount(dst, minlength=n_nodes).astype(np.float32)
    inv = (1.0 / np.maximum(counts, 1.0)).astype(np.float32)

    h_hi, h_lo = _bf16_split(h)
    h_split = np.concatenate([h_hi, h_lo], axis=1)  # [N, 256] bf16
    r_hi, r_lo = _bf16_split(r)

    core = dst // npc
    nl = dst % npc
    g = nl // 128
    is_hi = (src >= half).astype(np.int64)
    key = (core * G + g) * 2 + is_hi
    nkeys = n_cores * G * 2

    cnt = np.bincount(key, minlength=nkeys)
    chunks = -(-cnt // 128)
    caps = chunks.reshape(n_cores, G, 2).max(axis=0)  # [G, 2] chunks per run
    caps = np.maximum(caps, 1)

    run_chunk_base = np.zeros(G * 2, np.int64)
    flat = caps.reshape(-1)
    run_chunk_base[1:] = np.cumsum(flat)[:-1]
    run_chunk_base = run_chunk_base.reshape(G, 2)
    total_chunks = int(flat.sum())
    P_edges = total_chunks * 128

    order = np.argsort(key, kind="stable")
    key_s = key[order]
    run_starts = np.zeros(nkeys, np.int64)
    run_starts[1:] = np.cumsum(cnt)[:-1]
    offs = np.arange(E, dtype=np.int64) - run_starts[key_s]
    ecore = key_s // (G * 2)
    eg = (key_s // 2) % G
    ehalf = key_s % 2
    pos = run_chunk_base[eg, ehalf] * 128 + offs  # position in the padded stream

    per_core = []
    for c in range(n_cores):
        m = ecore == c
        p = pos[m]
        e = order[m]
        rT = np.zeros((128, total_chunks, 256), bf16)
        rT[p % 128, p // 128, 0:128] = r_hi[e]
        rT[p % 128, p // 128, 128:256] = r_lo[e]
        idx16 = np.zeros(P_edges, np.int16)
        idx16[p] = (src[e] - half * ehalf[m]).astype(np.int16)
        dstf = np.full((128, total_chunks), -1.0, np.float32)
        dstf[p % 128, p // 128] = (nl[e] - eg[m] * 128).astype(np.float32)
        idxw = np.tile(np.ascontiguousarray(idx16.reshape(-1, 16).T), (8, 1))
        invp = np.zeros(G * 128, np.float32)
        invp[:npc] = inv[c * npc : (c + 1) * npc]
        inv_t = np.ascontiguousarray(invp.reshape(G, 128).T)
        per_core.append(
            {
                "h": h_split,
                "rT": rT.reshape(128, total_chunks * 256),
                "idxw": np.ascontiguousarray(idxw),
                "dstf": dstf,
                "invt": inv_t,
                "iota": np.tile(np.arange(128, dtype=np.float32), (128, 1)).astype(bf16),
                "wt": np.ascontiguousarray(W.T),
                "bb": np.tile(b, (128, 1)),
                "ident": np.eye(128, dtype=np.float32),
            }
        )
    return per_core, caps, total_chunks, npc, G


def _build(caps, total_chunks, npc, G, n_nodes=N_NODES, half=HALF, gmax=8, nq=4):
    from contextlib import ExitStack

    import concourse.bacc as bacc
    import concourse.mybir as mybir
    import concourse.tile as tile

    f32 = mybir.dt.float32
    bf16 = mybir.dt.bfloat16
    i16 = mybir.dt.int16
    ADD = mybir.AluOpType.add
    MUL = mybir.AluOpType.mult
    ISEQ = mybir.AluOpType.is_equal

    Gmax = int(caps.sum(axis=1).max())

    nc = bacc.Bacc(
        "TRN2", target_bir_lowering=False, debug=False, num_swdge_queues=nq
    )
    h_d = nc.dram_tensor("h", [n_nodes, 256], bf16, kind="ExternalInput")
    r_d = nc.dram_tensor("rT", [128, total_chunks * 256], bf16, kind="ExternalInput")
    idx_d = nc.dram_tensor("idxw", [128, total_chunks * 8], i16, kind="ExternalInput")
    dstf_d = nc.dram_tensor("dstf", [128, total_chunks], f32, kind="ExternalInput")
    inv_d = nc.dram_tensor("invt", [128, G], f32, kind="ExternalInput")
    iota_d = nc.dram_tensor("iota", [128, 128], bf16, kind="ExternalInput")
    wt_d = nc.dram_tensor("wt", [128, 128], f32, kind="ExternalInput")
    bb_d = nc.dram_tensor("bb", [128, 128], f32, kind="ExternalInput")
    id_d = nc.dram_tensor("ident", [128, 128], f32, kind="ExternalInput")
    out_d = nc.dram_tensor("out", [npc, 128], f32, kind="ExternalOutput")

    h_lo_v = h_d[0:half, :]
    h_hi_v = h_d[half:n_nodes, :]

    with tile.TileContext(nc) as tc, ExitStack() as ctx:
        const = ctx.enter_context(tc.tile_pool(name="const", bufs=1))
        datap = ctx.enter_context(tc.tile_pool(name="data", bufs=8))
        idxp = ctx.enter_context(tc.tile_pool(name="idx", bufs=2))
        dstp = ctx.enter_context(tc.tile_pool(name="dstf", bufs=2))
        sp = ctx.enter_context(tc.tile_pool(name="s", bufs=8))
        ftp = ctx.enter_context(tc.tile_pool(name="ft", bufs=2))
        outp = ctx.enter_context(tc.tile_pool(name="o", bufs=2))
        psA = ctx.enter_context(tc.tile_pool(name="psA", bufs=2, space="PSUM"))
        psT = ctx.enter_context(tc.tile_pool(name="psT", bufs=2, space="PSUM"))
        psO = ctx.enter_context(tc.tile_pool(name="psO", bufs=2, space="PSUM"))

        iota_t = const.tile([128, 128], bf16)
        nc.sync.dma_start(iota_t[:], iota_d[:])
        wt_t = const.tile([128, 128], f32)
        nc.sync.dma_start(wt_t[:], wt_d[:])
        bb_t = const.tile([128, 128], f32)
        nc.sync.dma_start(bb_t[:], bb_d[:])
        id_t = const.tile([128, 128], f32)
        nc.sync.dma_start(id_t[:], id_d[:])
        inv_t = const.tile([128, G], f32)
        nc.sync.dma_start(inv_t[:], inv_d[:])

        cb = 0
        qctr = 0
        for g in range(G):
            ncl, nch = int(caps[g, 0]), int(caps[g, 1])
            ng = ncl + nch
            ix = idxp.tile([128, Gmax * 8], i16)
            nc.sync.dma_start(ix[:, : ng * 8], idx_d[:, cb * 8 : (cb + ng) * 8])
            df = dstp.tile([128, Gmax], f32)
            nc.sync.dma_start(df[:, :ng], dstf_d[:, cb : cb + ng])
            acc = psA.tile([128, 512], f32)
            done = 0
            for hf, ncap in ((0, ncl), (1, nch)):
                if ncap == 0:
                    continue
                hsrc = h_lo_v if hf == 0 else h_hi_v
                for q0 in range(0, ncap, gmax):
                    qn = min(gmax, ncap - q0)
                    c0 = done + q0
                    bt = datap.tile([128, 2, gmax, 256], bf16)
                    nc.gpsimd.dma_gather(
                        bt[:, 0, 0:qn, :],
                        hsrc,
                        ix[:, c0 * 8 : (c0 + qn) * 8],
                        qn * 128,
                        qn * 128,
                        256,
                        queue_num=qctr % nq,
                    )
                    qctr += 1
                    nc.sync.dma_start(
                        bt[:, 1, 0:qn, :],
                        r_d[:, (cb + c0) * 256 : (cb + c0 + qn) * 256].rearrange(
                            "p (j f) -> p j f", f=256
                        ),
                    )
                    for j in range(qn):
                        S = sp.tile([128, 128], bf16)
                        nc.vector.tensor_scalar(
                            S[:], iota_t[:], df[:, c0 + j : c0 + j + 1], None, ISEQ
                        )
                        nc.tensor.matmul(
                            acc[:],
                            S[:],
                            bt[:, :, j, :],
                            start=(c0 + j == 0),
                            stop=(c0 + j == ng - 1),
                        )
                done += ncap
            ft = ftp.tile([128, 128], f32)
            nc.vector.tensor_copy(ft[:], acc[:, 0:128])
            nc.vector.tensor_tensor(ft[:], ft[:], acc[:, 128:256], ADD)
            nc.vector.tensor_tensor(ft[:], ft[:], acc[:, 256:384], ADD)
            nc.vector.tensor_tensor(ft[:], ft[:], acc[:, 384:512], ADD)
            nc.vector.tensor_scalar(ft[:], ft[:], inv_t[:, g : g + 1], None, MUL)
            pt = psT.tile([128, 128], f32)
            nc.tensor.transpose(pt[:], ft[:], id_t[:])
            ftT = ftp.tile([128, 128], f32)
            nc.vector.tensor_copy(ftT[:], pt[:])
            po = psO.tile([128, 128], f32)
            nc.tensor.matmul(po[:], ftT[:], wt_t[:], start=True, stop=True)
            ot = outp.tile([128, 128], f32)
            nc.vector.tensor_tensor(ot[:], po[:], bb_t[:], ADD)
            rows = min(128, npc - g * 128)
            nc.sync.dma_start(out_d[g * 128 : g * 128 + rows, :], ot[0:rows, :])
            cb += ng

    nc.compile()
    return nc


LAST_RESULT = None


def kernel(src, dst, h, r, W, b, _trace=False, _tmpdir=None):
    global LAST_RESULT
    from concourse.bass_utils import run_bass_kernel_spmd

    per_core, caps, total_chunks, npc, G = _preprocess(src, dst, h, r, W, b)
    nc = _build(caps, total_chunks, npc, G)
    kwargs = {}
    if _trace:
        kwargs = dict(trace=True, tmpdir=_tmpdir)
    res = run_bass_kernel_spmd(nc, per_core, list(range(N_CORES)), **kwargs)
    LAST_RESULT = res
    out = np.concatenate([res.results[c]["out"] for c in range(N_CORES)], axis=0)
    return out.astype(np.float32)
